# revision 43
# baseline (speedup 1.0000x reference)
"""Bi-directional cross-attention kernel for Trainium2 (8 NeuronCores).

Strategy
--------
Sequence-parallel: core i owns query rows [i*512, (i+1)*512) of BOTH
attention directions and produces those rows of the final output. K/V
projections are recomputed on every core from the full h_2d/h_3d (cheaper
than an all-gather at these sizes), so no collectives are needed.

Math simplifications (done on host, exact):
  - K bias bk drops out of softmax (adds a per-query constant to every score).
  - V bias bv contributes bv @ Wo to every row (attn rows sum to 1), so it is
    folded into a single output-side bias bo_sum added before LayerNorm.
  - Q bias kept (applied during the Q^T PSUM->SBUF copy).

Device data layout (per core):
  - Everything transposed up-front on host: hT [din, n] so all matmuls are
    natural PE ops (contract over partitions).
  - Q^T, K^T stored [d_model, n] bf16.
  - V stored fp8e4m3 as [kv-chunk-pair, parity, head, 80] with a ones column
    at index 64: the fp8 DoubleRow AV matmul contracts TWO kv chunks per
    instruction and yields both O^T (rows 0-63) and the softmax denominator
    (row 64) in one accumulation.

Attention inner loop (the part that matters):
  - For each kv chunk k, BOTH heads of the current pair score into ONE
    2-bank PSUM tile: head-even via PE row-tile T0 (tile_position (0,0)),
    head-odd via T8 ((64,0)). The two contract-64 matmuls co-stream in
    disjoint halves of the PE array (~2x score throughput), because they
    share a single dependency (the previous exp of that tile) and so reach
    the PE queue back-to-back.
  - ONE ScalarE exp (FD=1024, scale=1/8 fused) converts the tile to fp8
    E[:, j, h, :] in SBUF. ScalarE is the roofline engine (~33.6M exp
    elements/core at 1 elem/cycle/partition); this structure keeps it
    saturated: scores for chunk k+1 depend only on the exp of chunk k-1's
    tag, two tags (sA/sB) alternate per kv parity.
  - fp8 DoubleRow AV contracts both kv chunks of a pair per instruction.
  - Projections (dir-1 Q/K/V during dir-0 attention, dir-0 out-proj during
    dir-1 attention) are emitted as deadline-scheduled PE filler.
"""

import numpy as np
import ml_dtypes

import concourse.bass as bass
import concourse.bacc as bacc
import concourse.mybir as mybir
import concourse.tile as tile
from concourse.bass_utils import run_bass_kernel_spmd

N = 4096
D2D, D3D, DM, H, DH = 256, 128, 512, 8, 64
NCORES = 8
NQ = N // NCORES          # 512 query rows per core
EPS = 1e-5
P = 128
KC = N // P               # 32 kv chunks of 128
QC = NQ // P              # 4 query chunks of 128
DMC = DM // P             # 4 d_model chunks of 128

BF = mybir.dt.bfloat16
F8 = mybir.dt.float8e4
F32 = mybir.dt.float32
AF = mybir.ActivationFunctionType
ALU = mybir.AluOpType

K2 = KC // 2     # 16 kv chunk-pairs (DoubleRow contracts 2 chunks per mm)
VW = 80          # padded per-head V row width in fp8 (step % 16 == 0)

TRACE = False
_cache = {}


def _build_program(gb_identity):
    # Bacc (not bare Bass): its compile() splits multi-semaphore waits into
    # standalone event-semaphore instructions (TRN2 allows 1 wait per inst).
    nc = bacc.Bacc("TRN2", target_bir_lowering=False)

    # ---- I/O -----------------------------------------------------------
    # all big tensors are packed [P, o*f] on host so each SBUF partition's
    # data is one contiguous HBM chunk -> one DMA descriptor per partition
    hq2dT = nc.dram_tensor("hq2dT", [P, 2 * NQ], BF, kind="ExternalInput")
    hq3dT = nc.dram_tensor("hq3dT", [P, 1 * NQ], BF, kind="ExternalInput")
    hkv2dT = nc.dram_tensor("hkv2dT", [P, 2 * N], BF, kind="ExternalInput")
    hkv3dT = nc.dram_tensor("hkv3dT", [P, 1 * N], BF, kind="ExternalInput")
    Wq2d = nc.dram_tensor("Wq2d", [P, 2 * DM], BF, kind="ExternalInput")
    Wk3d = nc.dram_tensor("Wk3d", [P, 1 * DM], BF, kind="ExternalInput")
    Wv3d = nc.dram_tensor("Wv3d", [P, 1 * DM], BF, kind="ExternalInput")
    Wq3d = nc.dram_tensor("Wq3d", [P, 1 * DM], BF, kind="ExternalInput")
    Wk2d = nc.dram_tensor("Wk2d", [P, 2 * DM], BF, kind="ExternalInput")
    Wv2d = nc.dram_tensor("Wv2d", [P, 2 * DM], BF, kind="ExternalInput")
    Wo23 = nc.dram_tensor("Wo23", [P, 4 * DM], BF, kind="ExternalInput")
    Wo32 = nc.dram_tensor("Wo32", [P, 4 * DM], BF, kind="ExternalInput")
    bq2dT = nc.dram_tensor("bq2dT", [P, 4], F32, kind="ExternalInput")
    bq3dT = nc.dram_tensor("bq3dT", [P, 4], F32, kind="ExternalInput")
    bo_sum = nc.dram_tensor("bo_sum", [1, DM], F32, kind="ExternalInput")
    gamma_r = nc.dram_tensor("gamma_r", [1, DM], F32, kind="ExternalInput")
    beta_r = nc.dram_tensor("beta_r", [1, DM], F32, kind="ExternalInput")
    out = nc.dram_tensor("out", [NQ, DM], F32, kind="ExternalOutput")

    with tile.TileContext(nc) as tc:
        with (
            tc.tile_pool(name="const", bufs=1) as const,
            tc.tile_pool(name="kv", bufs=1) as kv,
            tc.tile_pool(name="epool", bufs=3) as epool,
            tc.tile_pool(name="rpool", bufs=2) as rpool,
            tc.tile_pool(name="misc", bufs=2) as misc,
            tc.tile_pool(name="psum", bufs=1, space="PSUM") as psum,
        ):
            # ---- constants / weights into SBUF -------------------------
            # each dma_start lands on its own queue; `splits` stripes a big
            # tensor over several queues so the head phase is not gated on
            # one queue's descriptor rate
            # one dma_start per tensor (each costs ~650ns of serial
            # sequencer dispatch), with the attention-critical loads'
            # triggers spread across idle engine queues so their dispatch
            # runs in parallel rather than serially on Sync
            def load(dram, shape, dtype=BF, engs=None):
                # each dma_start's descriptors land on ONE HW queue at ~1
                # descriptor (partition) per ~80-230ns, so a load used soon
                # must be split by partition range across several calls;
                # trigger engines alternate to parallelize the ~650ns
                # per-call sequencer dispatch as well
                t = const.tile(shape, dtype, name=dram.name + "_sb")
                src = dram[:].rearrange("p (o f) -> p o f", o=shape[1])
                if not engs:
                    engs = [nc.sync]
                step = P // len(engs)
                for s, eng in enumerate(engs):
                    rows = slice(s * step, (s + 1) * step)
                    eng.dma_start(t[rows], src[rows])
                return t

            # dir-0 attention-critical operands first, in dependency order.
            # Dispatch only on sync+gpsimd: the scalar queue must stay clear
            # so the activation-table load and the first Q-bias Identity can
            # run as soon as their data lands.
            bq1 = const.tile([P, 4], F32, name="bq1")
            nc.sync.dma_start(bq1, bq2dT[:])
            wq1 = load(Wq2d, [P, 2, DM], engs=[nc.sync, nc.gpsimd])
            hq1 = load(hq2dT, [P, 2, NQ], engs=[nc.gpsimd, nc.sync])
            wk1 = load(Wk3d, [P, 1, DM], engs=[nc.sync, nc.gpsimd])
            wv1 = load(Wv3d, [P, 1, DM], engs=[nc.gpsimd, nc.sync])
            # hkv1 split by column need: the first kv columns gate the
            # prologue; the rest is only consumed from iteration ~4 on
            hkv1 = const.tile([P, 1, N], BF, name="hkv3dT_sb")
            hkv1_src = hkv3dT[:].rearrange("p (o f) -> p o f", o=1)
            for s, eng in enumerate([nc.sync, nc.gpsimd]):
                nc_rows = slice(s * 64, (s + 1) * 64)
                eng.dma_start(hkv1[nc_rows, :, 0:1024],
                              hkv1_src[nc_rows, :, 0:1024])
            for s, eng in enumerate([nc.gpsimd, nc.sync]):
                nc_rows = slice(s * 64, (s + 1) * 64)
                eng.dma_start(hkv1[nc_rows, :, 1024:N],
                              hkv1_src[nc_rows, :, 1024:N])
            bq2 = const.tile([P, 4], F32, name="bq2")
            nc.sync.dma_start(bq2, bq3dT[:])
            wq2 = load(Wq3d, [P, 1, DM])
            hq2 = load(hq3dT, [P, 1, NQ])
            wk2 = load(Wk2d, [P, 2, DM])
            wv2 = load(Wv2d, [P, 2, DM])
            hkv2 = load(hkv2dT, [P, 2, N])
            wo1 = load(Wo23, [P, 4, DM])
            wo2 = load(Wo32, [P, 4, DM])

            bo_bc = const.tile([P, DM], F32, name="bo_bc")
            nc.sync.dma_start(bo_bc, bo_sum[:].to_broadcast([P, DM]))
            if not gb_identity:
                gamma_bc = const.tile([P, DM], F32, name="gamma_bc")
                nc.sync.dma_start(gamma_bc, gamma_r[:].to_broadcast([P, DM]))
                beta_bc = const.tile([P, DM], F32, name="beta_bc")
                nc.sync.dma_start(beta_bc, beta_r[:].to_broadcast([P, DM]))
            eps_sb = const.tile([P, 1], F32, name="eps_sb")
            nc.vector.memset(eps_sb, EPS)

            x1 = kv.tile([P, QC, DM], F32, name="x1")
            kt1 = kv.tile([P, DMC, N], BF, name="kt1")
            vv1 = kv.tile([P, K2, 2, H, VW], F8, name="vv1")
            qt1 = kv.tile([P, DMC, NQ], BF, name="qt1")
            ot1 = kv.tile([P, DMC, NQ], BF, name="ot1")
            kt2 = kv.tile([P, DMC, N], BF, name="kt2")
            vv2 = kv.tile([P, K2, 2, H, VW], F8, name="vv2")
            qt2 = kv.tile([P, DMC, NQ], BF, name="qt2")
            ot2 = kv.tile([P, DMC, NQ], BF, name="ot2")

            # ones columns for the AV denominator rows, once per direction
            nc.vector.memset(vv1[:, :, :, :, DH:DH + 1], 1.0)
            nc.vector.memset(vv2[:, :, :, :, DH:DH + 1], 1.0)

            dirs = [
                dict(hq=hq1, nq=2, hkv=hkv1, nk=1, wq=wq1, wk=wk1, wv=wv1,
                     wo=wo1, bq=bq1, kt=kt1, vv=vv1, qt=qt1, ot=ot1),
                dict(hq=hq2, nq=1, hkv=hkv2, nk=2, wq=wq2, wk=wk2, wv=wv2,
                     wo=wo2, bq=bq2, kt=kt2, vv=vv2, qt=qt2, ot=ot2),
            ]

            # ---- projection emitters (PSUM tag "pj", double-buffered) --
            def pj():
                return psum.tile([P, 512], F32, tag="pj", bufs=2, name="pj")

            def emit_qt(D, c):
                ps = pj()
                for dc in range(D["nq"]):
                    nc.tensor.matmul(
                        ps, lhsT=D["wq"][:, dc, c * P:(c + 1) * P],
                        rhs=D["hq"][:, dc, :],
                        start=(dc == 0), stop=(dc == D["nq"] - 1))
                # bias-add on ScalarE (a DVE tensor_scalar would need 2 sem
                # waits, exceeding its single ISA wait slot)
                nc.scalar.activation(
                    out=D["qt"][:, c, :], in_=ps, func=AF.Identity,
                    bias=D["bq"][:, c:c + 1])

            def emit_kt(D, c, f):
                ps = pj()
                for dc in range(D["nk"]):
                    nc.tensor.matmul(
                        ps, lhsT=D["wk"][:, dc, c * P:(c + 1) * P],
                        rhs=D["hkv"][:, dc, f * 512:(f + 1) * 512],
                        start=(dc == 0), stop=(dc == D["nk"] - 1))
                nc.vector.tensor_copy(
                    out=D["kt"][:, c, f * 512:(f + 1) * 512], in_=ps)

            def emit_v(D, k):
                ps = pj()
                for dc in range(D["nk"]):
                    nc.tensor.matmul(
                        ps, lhsT=D["hkv"][:, dc, k * P:(k + 1) * P],
                        rhs=D["wv"][:, dc, :],
                        start=(dc == 0), stop=(dc == D["nk"] - 1))
                nc.vector.tensor_copy(
                    out=D["vv"][:, k // 2, k % 2, :, 0:DH],
                    in_=ps.rearrange("p (h e) -> p h e", h=H))

            # progressive out-projection: after each head-pair's normalize,
            # its d_model chunk's contribution to BOTH directions' output
            # projections is matmul'd and accumulated into xacc in SBUF, so
            # only the final pair's chunk + LayerNorm remain at the tail
            xacc_init = [False] * QC

            def emit_partial(D, p, q):
                ps = pj()
                nc.tensor.matmul(
                    ps, lhsT=D["ot"][:, p, q * P:(q + 1) * P],
                    rhs=D["wo"][:, p, :], start=True, stop=True)
                prev = bo_bc if not xacc_init[q] else x1[:, q, :]
                xacc_init[q] = True
                nc.vector.tensor_add(out=x1[:, q, :], in0=ps, in1=prev)

            def emit_final(q):
                D = dirs[1]
                ps = pj()
                nc.tensor.matmul(
                    ps, lhsT=D["ot"][:, DMC - 1, q * P:(q + 1) * P],
                    rhs=D["wo"][:, DMC - 1, :], start=True, stop=True)
                # x = partials + last chunk ; LayerNorm ; write out
                x_t = misc.tile([P, DM], F32, tag="x_t", bufs=3, name="x_t")
                nc.vector.tensor_add(out=x_t, in0=ps, in1=x1[:, q, :])
                stats = misc.tile([P, 6], F32, tag="stats", name="stats")
                nc.vector.bn_stats(out=stats, in_=x_t)
                mv = misc.tile([P, 2], F32, tag="mv", name="mv")
                nc.vector.bn_aggr(out=mv, in_=stats)
                std = misc.tile([P, 1], F32, tag="std", name="std")
                nc.scalar.activation(out=std, in_=mv[:, 1:2],
                                     func=AF.Sqrt, bias=eps_sb[:, 0:1])
                rstd = misc.tile([P, 1], F32, tag="rstd", name="rstd")
                nc.vector.reciprocal(rstd, std)
                nc.vector.tensor_scalar(
                    out=x_t, in0=x_t, scalar1=mv[:, 0:1],
                    scalar2=rstd, op0=ALU.subtract, op1=ALU.mult)
                if not gb_identity:
                    # skipped when the actual inputs have gamma==1, beta==0
                    # (checked at build time) — then these are exact no-ops
                    nc.vector.tensor_mul(out=x_t, in0=x_t, in1=gamma_bc)
                    nc.vector.tensor_add(out=x_t, in0=x_t, in1=beta_bc)
                # store trigger on a rotating engine queue: each dma_start
                # costs ~650ns of sequencer dispatch, serial per engine
                # split every store by partition across parallel queues (a
                # single call's 128 descriptors serialize on one HW queue
                # and would pile up behind the other chunks' stores)
                for s, eng in enumerate(
                        [nc.sync, nc.gpsimd, nc.sync, nc.gpsimd]):
                    eng.dma_start(
                        out[q * P + s * 32:q * P + (s + 1) * 32, :],
                        x_t[s * 32:(s + 1) * 32, :])

            # deferred normalize + partial-outproj closures, drained one per
            # k2 iteration (granular: no multi-us DVE blobs that would
            # head-of-line-block the casts feeding the PE filler)
            pending = []

            def norm_recip(ou):
                # 1/rowsum via 32x32 block-transpose so the iterative-divide
                # reciprocal runs on FD=16 instead of FD=512 (3.3us -> ~0.3us)
                tr = rpool.tile([32, 512], F32, tag="trD", name="trD")
                nc.vector.transpose(tr, ou[DH:DH + 32, :])
                rt = rpool.tile([32, 512], F32, tag="rt", name="rt")
                tr3 = tr.rearrange("p (b j) -> p b j", j=32)
                rt3 = rt.rearrange("p (b j) -> p b j", j=32)
                nc.vector.reciprocal(rt3[:, :, 0:1], tr3[:, :, 0:1])
                # transpose back into the trD buffer (recip already read it)
                nc.vector.transpose(tr, rt)
                return tr

            def norm_apply(D, c, i, ou, tr):
                ot = D["ot"]
                po = i * DH
                rbc = rpool.tile([DH, 512], F32, tag="r_bc", bufs=1,
                                 name="rbc")
                nc.gpsimd.partition_broadcast(rbc, tr[0:1, :])
                if po == 0:
                    nc.vector.tensor_mul(
                        out=ot[0:DH, c, :], in0=ou[0:DH, :], in1=rbc)
                else:
                    # DVE cannot shift partitions: stage at base 0, then
                    # DMA into partitions 64-127
                    stg = rpool.tile([DH, 512], BF, tag="ot_stage",
                                     name="stg")
                    nc.vector.tensor_mul(
                        out=stg, in0=ou[0:DH, :], in1=rbc)
                    nc.sync.dma_start(ot[po:po + DH, c, :], stg)

            def norm_head(D, c, i, ou):
                norm_apply(D, c, i, ou, norm_recip(ou))

            # ---- attention for one direction, with PE filler -----------
            # prev_av carries ACROSS pair (and direction) boundaries: the
            # last AV of pair p and the PSUM evacuation are emitted inside
            # pair p+1's first iterations, so pair p+1's first scores+exps
            # never queue behind them
            av_state = {"prev": None, "evac": None}

            def attention(d, D, filler):
                kt, vv, qt, ot = D["kt"], D["vv"], D["qt"], D["ot"]
                for pair in range(H // 2):
                    c = pair
                    o_ps = [psum.tile([P, 512], F32, tag=f"o{i}", bufs=1,
                                      name=f"o_ps{i}") for i in range(2)]
                    for k2 in range(K2):
                        e_t = epool.tile([P, 2, 2, 512], F8, tag="E",
                                         name="e_t")
                        for j, stag in ((0, "sA"), (1, "sB")):
                            k = 2 * k2 + j
                            s = psum.tile([P, 1024], F32, tag=stag, bufs=1,
                                          name=stag)
                            # both heads of the pair score into one 2-bank
                            # tile via disjoint PE row-tiles; the pair shares
                            # one dependency so the matmuls co-stream
                            nc.tensor.matmul(
                                s[:, 0:512],
                                lhsT=kt[0:DH, c, k * P:(k + 1) * P],
                                rhs=qt[0:DH, c, :],
                                start=True, stop=True, tile_position=(0, 0))
                            nc.tensor.matmul(
                                s[:, 512:1024],
                                lhsT=kt[DH:P, c, k * P:(k + 1) * P],
                                rhs=qt[DH:P, c, :],
                                start=True, stop=True, tile_position=(64, 0))
                            nc.scalar.activation(
                                out=e_t[:, j],
                                in_=s.rearrange("p (h q) -> p h q", h=2),
                                func=AF.Exp, scale=0.125)
                        # emit the PREVIOUS chunk's AV after this chunk's
                        # scores so the scores win the PE priority race and
                        # the next exp is never gated behind the AVs
                        if av_state["prev"] is not None:
                            av_state["prev"]()
                        if av_state["evac"] is not None:
                            av_state["evac"]()
                            av_state["evac"] = None
                        def make_av(k2=k2, e_t=e_t, o_ps=o_ps, pair=pair,
                                    vv=vv):
                            def emit():
                                for i in range(2):
                                    # fp8 DoubleRow AV: contracts both kv
                                    # chunks per matmul; vv col 64 is ones so
                                    # row 64 accumulates the denominator
                                    nc.tensor.matmul(
                                        o_ps[i][0:DH + 1, :],
                                        lhsT=vv[:, k2, :, 2 * pair + i,
                                                0:DH + 1],
                                        rhs=e_t[:, :, i, :],
                                        start=(k2 == 0), stop=(k2 == K2 - 1),
                                        perf_mode=(
                                            mybir.MatmulPerfMode.DoubleRow),
                                        skip_group_check=True)
                            return emit
                        av_state["prev"] = make_av()
                        # drain deferred work every OTHER iteration: these
                        # closures have huge slack, and packing them densely
                        # lets their late-ready adds clog the engine FIFOs
                        if k2 >= 2 and k2 % 2 == 0 and pending:
                            pending.pop(0)()
                        filler()

                    # PSUM evacuation + deferred normalize/partials for this
                    # pair, emitted inside the NEXT pair's first iteration
                    def make_evac(D=D, c=c, pair=pair, o_ps=o_ps):
                        def emit():
                            ous = []
                            for i in range(2):
                                ou = rpool.tile([96, 512], F32, tag="ou",
                                                name="ou")
                                nc.vector.tensor_copy(
                                    out=ou[0:DH + 1, :],
                                    in_=o_ps[i][0:DH + 1, :])
                                ous.append(ou)
                            pending.extend(
                                [(lambda i=i, ou=ous[i]:
                                  norm_head(D, c, i, ou))
                                 for i in range(2)]
                                + [(lambda q=q: emit_partial(D, pair, q))
                                   for q in range(QC)])
                        return emit
                    last_pair = d == 1 and pair == H // 2 - 1
                    if last_pair:
                        av_state["prev"]()
                        av_state["prev"] = None
                        for fn in pending:
                            fn()
                        pending.clear()
                        ous = []
                        for i in range(2):
                            ou = rpool.tile([96, 512], F32, tag="ou",
                                            name="ou")
                            nc.vector.tensor_copy(out=ou[0:DH + 1, :],
                                                  in_=o_ps[i][0:DH + 1, :])
                            ous.append(ou)
                        # interleave the two heads' normalize chains, odd
                        # head first so its ot DMA (which gates the final
                        # matmuls) issues as early as possible
                        trs = {i: norm_recip(ous[i]) for i in (1, 0)}
                        for i in (1, 0):
                            norm_apply(D, c, i, ous[i], trs[i])
                        for q in range(QC):
                            emit_final(q)
                    else:
                        av_state["evac"] = make_evac()

            # ---- emission schedule -------------------------------------
            D0, D1 = dirs

            # minimal dir-0 prologue: just enough for attention pair 0 to
            # start (qt chunk 0, first half of kt chunk 0, first 8 v chunks)
            emit_qt(D0, 0)
            for f in range(4):
                emit_kt(D0, 0, f)
            for k in range(8):
                emit_v(D0, k)

            # everything else becomes deadline-scheduled PE filler inside
            # dir-0 attention: (need-by iteration, task)
            tasks = []
            for f in range(4, 8):
                tasks.append((2 * f - 3, ("kt", D0, 0, f)))
            for k in range(8, KC):
                tasks.append((max(0, k // 2 - 3), ("v", D0, k)))
            for c in range(1, DMC):
                tasks.append((16 * c - 6, ("qt", D0, c)))
                for f in range(8):
                    tasks.append((16 * c + 2 * f - 5, ("kt", D0, c, f)))
            # early dir-1 projections spread over iterations 28..58 of dir-0
            # attention; the LATE-needed half (kv chunks 16+, kt chunks 2-3)
            # moves into dir-1's own attention, whose PE has spare slots
            d1 = ([("qt", D1, c) for c in range(DMC)]
                  + [("kt", D1, c, f) for c in range(2) for f in range(8)]
                  + [("v", D1, k) for k in range(16)])
            for idx, t in enumerate(d1):
                tasks.append((28 + (idx * 30) // len(d1), t))
            tasks.sort(key=lambda t: t[0])

            tasks1 = ([(max(0, k // 2 - 3), ("v", D1, k))
                       for k in range(16, KC)]
                      + [(16 * c + 2 * f - 5, ("kt", D1, c, f))
                         for c in range(2, DMC) for f in range(8)])
            tasks1.sort(key=lambda t: t[0])

            def run_task(t):
                if t[0] == "qt":
                    emit_qt(t[1], t[2])
                elif t[0] == "kt":
                    emit_kt(t[1], t[2], t[3])
                else:
                    emit_v(t[1], t[2])

            n_iters = (H // 2) * K2
            state0 = {"done": 0, "calls": 0}

            def filler0():
                it = state0["calls"]
                state0["calls"] += 1
                want = (len(tasks) * (it + 1)) // n_iters
                while state0["done"] < len(tasks) and (
                        tasks[state0["done"]][0] <= it
                        or state0["done"] < want):
                    run_task(tasks[state0["done"]][1])
                    state0["done"] += 1

            attention(0, D0, filler0)
            while state0["done"] < len(tasks):
                run_task(tasks[state0["done"]][1])
                state0["done"] += 1

            state1 = {"done": 0, "calls": 0}

            def filler1():
                it = state1["calls"]
                state1["calls"] += 1
                want = (len(tasks1) * (it + 1)) // n_iters
                while state1["done"] < len(tasks1) and (
                        tasks1[state1["done"]][0] <= it
                        or state1["done"] < want):
                    run_task(tasks1[state1["done"]][1])
                    state1["done"] += 1

            # dir-1 attention; the tail (final chunk out-proj + LayerNorm)
            # is emitted inside attention() for the last pair
            attention(1, D1, filler1)

    nc.compile()
    return nc


def _prep_inputs(inputs):
    bf = ml_dtypes.bfloat16
    f = {k: np.asarray(v, dtype=np.float32) for k, v in inputs.items()}

    h2dT = np.ascontiguousarray(f["h_2d"].T).astype(bf)      # [256, 4096]
    h3dT = np.ascontiguousarray(f["h_3d"].T).astype(bf)      # [128, 4096]

    def pack(a):
        # [o*P, X] -> [P, o*X]: partition p's chunks made contiguous
        o = a.shape[0] // P
        return np.ascontiguousarray(
            a.reshape(o, P, -1).transpose(1, 0, 2).reshape(P, -1))

    def wchunk(w):
        return pack(np.asarray(w, dtype=np.float32).astype(bf))

    bo = (f["bo23"].astype(np.float64)
          + f["bv3d"].astype(np.float64) @ f["Wo23"].astype(np.float64)
          + f["bo32"].astype(np.float64)
          + f["bv2d"].astype(np.float64) @ f["Wo32"].astype(np.float64))

    common = {
        "hkv2dT": pack(h2dT),
        "hkv3dT": pack(h3dT),
        "Wq2d": wchunk(f["Wq2d"]), "Wk3d": wchunk(f["Wk3d"]),
        "Wv3d": wchunk(f["Wv3d"]), "Wq3d": wchunk(f["Wq3d"]),
        "Wk2d": wchunk(f["Wk2d"]), "Wv2d": wchunk(f["Wv2d"]),
        "Wo23": wchunk(f["Wo23"]), "Wo32": wchunk(f["Wo32"]),
        "bq2dT": np.ascontiguousarray(f["bq2d"].reshape(4, P).T),
        "bq3dT": np.ascontiguousarray(f["bq3d"].reshape(4, P).T),
        "bo_sum": np.ascontiguousarray(bo.astype(np.float32)[None, :]),
        "gamma_r": np.ascontiguousarray(f["gamma"][None, :]),
        "beta_r": np.ascontiguousarray(f["beta"][None, :]),
    }

    in_maps = []
    for i in range(NCORES):
        sl = slice(i * NQ, (i + 1) * NQ)
        m = dict(common)
        m["hq2dT"] = pack(np.ascontiguousarray(h2dT[:, sl]))
        m["hq3dT"] = pack(np.ascontiguousarray(h3dT[:, sl]))
        in_maps.append(m)
    return in_maps


def kernel(**inputs) -> np.ndarray:
    gb_identity = bool(
        np.allclose(np.asarray(inputs["gamma"], dtype=np.float32), 1.0)
        and np.allclose(np.asarray(inputs["beta"], dtype=np.float32), 0.0))
    key = ("nc", gb_identity)
    if key not in _cache:
        _cache[key] = _build_program(gb_identity)
    nc = _cache[key]
    in_maps = _prep_inputs(inputs)
    res = run_bass_kernel_spmd(nc, in_maps, core_ids=list(range(NCORES)),
                               trace=TRACE)
    _cache["last_result"] = res
    return np.concatenate([r["out"] for r in res.results], axis=0)
